# revision 7
# baseline (speedup 1.0000x reference)
"""VQ codebook kernel (DeepECT) for 8 Trainium2 NeuronCores.

Problem: z [262144, 64] f32, centers [256, 64] f32, weights [256] f32.
Returns (cos_dist [N] f32, new_weights [256] f32, assignments [N] int32)
matching the reference:
    d2        = ||z||^2 - 2 z@C^T + ||c||^2
    a         = argmin_k d2
    counts    = bincount(a);  new_w = 0.5 w + 0.5 counts/N
    cos_dist  = 1 - (z . c_a) / (||z|| ||c_a||)

Strategy (data-parallel over N across 8 cores, everything heavy on-device):
  host:  zT_aug[c] = [z_shard.T ; ones]  (65 x 32768)  per core
         CT_aug = [centers.T ; -0.5*||c||^2]  (65 x 256)
  core:  t = zT_aug.T @ CT_aug    (PE fp32; argmax_k t == argmin_k d2;
                                   t[n,k] = z.c_k - csq_k/2)
         max_t = reduce_max(t)                       (DVE, batched, PSUM)
         oh    = (t >= max_t)                        (DVE, batched, PSUM)
         idx   = 256 - max_k(oh * (256-k))           (DVE TTR per tile)
         csq_a = sum_k(oh * csq_k)                   (DVE TTR per tile)
         counts= ones.T @ oh  (PE, PSUM-accumulated over all tiles)
         zsq   = ones.T @ square(zT)                 (ACT + PE f32r)
         cos   = 1 - (max_t + csq_a/2) * sqrt(1/(zsq*csq_a))
  host:  unshard, sum per-core counts, EMA update of weights.
"""

import os
import sys

sys.path.insert(0, "/opt/trn_rl_repo")

import numpy as np

import concourse.bacc as bacc
import concourse.mybir as mybir
import concourse.tile as tile
from concourse.bass_utils import run_bass_kernel_spmd

F32 = mybir.dt.float32
F32R = mybir.dt.float32r
I32 = mybir.dt.int32
AX = mybir.AxisListType
ALU = mybir.AluOpType
ACTF = mybir.ActivationFunctionType

N, D, K = 262144, 64, 256
NCORES = 8
NPC = N // NCORES          # samples per core
TILE = 128                 # samples per matmul tile (PSUM partitions)
GTILES = 4                 # tiles per PSUM group (2 banks of fp32)
GSAMP = TILE * GTILES      # 512 samples per group
ALPHA = 0.5


def build_nc(ngroups=NPC // GSAMP):
    """Build the per-core SPMD program. ngroups*512 samples are processed."""
    npc = ngroups * GSAMP
    ntiles = ngroups * GTILES

    nc = bacc.Bacc("TRN2", num_devices=NCORES, debug=False)

    zt_d = nc.dram_tensor("zt", [D + 1, npc], F32, kind="ExternalInput")
    ct_d = nc.dram_tensor("ct", [D + 1, K], F32, kind="ExternalInput")
    csqrep_d = nc.dram_tensor("csqrep", [TILE, K], F32, kind="ExternalInput")
    revkrep_d = nc.dram_tensor("revkrep", [TILE, K], F32, kind="ExternalInput")

    cosd_d = nc.dram_tensor("cosd", [TILE, ntiles], F32, kind="ExternalOutput")
    idx_d = nc.dram_tensor("idxout", [TILE, ntiles], I32, kind="ExternalOutput")
    cnt_d = nc.dram_tensor("cnt", [1, K], F32, kind="ExternalOutput")

    with tile.TileContext(nc) as tc:
        with (
            tc.tile_pool(name="tables", bufs=1) as tabp,
            tc.tile_pool(name="persist", bufs=1) as perp,
            tc.tile_pool(name="zchunk", bufs=3) as zp,
            tc.tile_pool(name="sqchunk", bufs=2) as sqp,
            tc.tile_pool(name="oh", bufs=2) as ohp,
            tc.tile_pool(name="trash", bufs=4) as trp,
            tc.tile_pool(name="ps", bufs=2, space="PSUM") as psp,
            tc.tile_pool(name="zq", bufs=2, space="PSUM") as zqp,
            tc.tile_pool(name="cntps", bufs=1, space="PSUM") as cpsp,
        ):
            # --- constant tables ---
            ct_sb = tabp.tile([D + 1, K], F32, tag="ct")
            nc.sync.dma_start(ct_sb[:], ct_d[:])
            csqrep = tabp.tile([TILE, K], F32, tag="csqrep")
            nc.sync.dma_start(csqrep[:], csqrep_d[:])
            revkrep = tabp.tile([TILE, K], F32, tag="revkrep")
            nc.sync.dma_start(revkrep[:], revkrep_d[:])
            ones_cnt = tabp.tile([TILE, 1], F32R, tag="ones_cnt")
            ones_f = tabp.tile([TILE, 1], F32, tag="ones_f")
            nc.gpsimd.memset(ones_f[:], 1.0)
            nc.vector.tensor_copy(ones_cnt[:], ones_f[:])
            ones_sq = tabp.tile([D, 1], F32, tag="ones_sq")
            nc.gpsimd.memset(ones_sq[:], 1.0)

            # --- persistent per-sample buffers [128, ntiles] ---
            maxt_sb = perp.tile([TILE, ntiles], F32, tag="maxt")
            idxa_sb = perp.tile([TILE, ntiles], F32, tag="idxa")
            csqa_sb = perp.tile([TILE, ntiles], F32, tag="csqa")
            zsq_sb = perp.tile([TILE, ntiles], F32, tag="zsq")

            counts_ps = cpsp.tile([1, K], F32, tag="cnt")

            for g in range(ngroups):
                zchunk = zp.tile([D + 1, GSAMP], F32)
                nc.sync.dma_start(
                    zchunk[:], zt_d[:, g * GSAMP : (g + 1) * GSAMP]
                )

                # squares for ||z||^2
                sqchunk = sqp.tile([D, GSAMP], F32)
                nc.scalar.activation(sqchunk[:], zchunk[0:D, :], ACTF.Square)

                ps = psp.tile([TILE, GTILES * K], F32)
                for i in range(GTILES):
                    nc.tensor.matmul(
                        ps[:, i * K : (i + 1) * K],
                        zchunk[:, i * TILE : (i + 1) * TILE],
                        ct_sb[:],
                        start=True,
                        stop=True,
                    )

                # zsq columns: lhsT = squares tile (samples on M), rhs = ones
                zq = zqp.tile([TILE, GTILES], F32)
                for i in range(GTILES):
                    nc.tensor.matmul(
                        zq[:, i : i + 1],
                        sqchunk[:, i * TILE : (i + 1) * TILE],
                        ones_sq[:],
                        start=True,
                        stop=True,
                    )
                nc.scalar.copy(
                    zsq_sb[:, g * GTILES : (g + 1) * GTILES], zq[:]
                )

                # batched max over k for the whole group
                ps3 = ps[:].rearrange("p (i k) -> p i k", i=GTILES, k=K)
                mslice = maxt_sb[:, g * GTILES : (g + 1) * GTILES]
                nc.vector.tensor_reduce(mslice, ps3, axis=AX.X, op=ALU.max)

                # fused select+gather per tile:
                #   prod = (t >= max_t) * revk ; accum = 256 - argmax  (unique max)
                #   tr   = (t >= max_t) * csq  ; accum = csq[argmax]
                prod = ohp.tile([TILE, GTILES * K], F32R, tag="prod")
                for i in range(GTILES):
                    t = g * GTILES + i
                    tps = ps[:, i * K : (i + 1) * K]
                    mcol = maxt_sb[:, t : t + 1]
                    nc.vector.scalar_tensor_tensor(
                        out=prod[:, i * K : (i + 1) * K],
                        in0=tps,
                        scalar=mcol,
                        in1=revkrep[:],
                        op0=ALU.is_ge,
                        op1=ALU.mult,
                        accum_out=idxa_sb[:, t : t + 1],
                    )
                    tr2 = trp.tile([TILE, K], F32R, tag="tr2")
                    nc.vector.scalar_tensor_tensor(
                        out=tr2[:],
                        in0=tps,
                        scalar=mcol,
                        in1=csqrep[:],
                        op0=ALU.is_ge,
                        op1=ALU.mult,
                        accum_out=csqa_sb[:, t : t + 1],
                    )
                    # counts (scaled by revk; host divides): ones.T @ prod
                    nc.tensor.matmul(
                        counts_ps[:],
                        ones_cnt[:],
                        prod[:, i * K : (i + 1) * K],
                        start=(t == 0),
                        stop=(t == ntiles - 1),
                    )

            # ---- epilogue: per-sample math on [128, ntiles] ----
            half = perp.tile([TILE, ntiles], F32, tag="half")
            nc.vector.tensor_scalar_mul(half[:], csqa_sb[:], 0.5)
            dot = perp.tile([TILE, ntiles], F32, tag="dot")
            nc.vector.tensor_add(dot[:], maxt_sb[:], half[:])
            prod = perp.tile([TILE, ntiles], F32, tag="prod")
            nc.vector.tensor_mul(prod[:], zsq_sb[:], csqa_sb[:])
            rec = perp.tile([TILE, ntiles], F32, tag="rec")
            nc.vector.reciprocal(rec[:], prod[:])
            root = perp.tile([TILE, ntiles], F32, tag="root")
            nc.scalar.activation(root[:], rec[:], ACTF.Sqrt)
            cosn = perp.tile([TILE, ntiles], F32, tag="cosn")
            nc.vector.tensor_mul(cosn[:], dot[:], root[:])
            cosf = perp.tile([TILE, ntiles], F32, tag="cosf")
            nc.vector.tensor_scalar(
                cosf[:], cosn[:], -1.0, 1.0, op0=ALU.mult, op1=ALU.add
            )
            nc.sync.dma_start(cosd_d[:], cosf[:])

            idxf = perp.tile([TILE, ntiles], F32, tag="idxf")
            nc.vector.tensor_scalar(
                idxf[:], idxa_sb[:], -1.0, 256.0, op0=ALU.mult, op1=ALU.add
            )
            idxi = perp.tile([TILE, ntiles], I32, tag="idxi")
            nc.vector.tensor_copy(idxi[:], idxf[:])
            nc.sync.dma_start(idx_d[:], idxi[:])

            cnt_sb = perp.tile([1, K], F32, tag="cntsb")
            nc.scalar.copy(cnt_sb[:], counts_ps[:])
            nc.sync.dma_start(cnt_d[:], cnt_sb[:])

    nc.compile()
    return nc


_NC_CACHE = {}


def _get_nc():
    if "nc" not in _NC_CACHE:
        _NC_CACHE["nc"] = build_nc()
    return _NC_CACHE["nc"]


def make_inputs(z, centers, weights, npc=NPC):
    """Host-side layout prep -> per-core input maps."""
    z = np.asarray(z, dtype=np.float32)
    centers = np.asarray(centers, dtype=np.float32)
    csq = np.sum(centers * centers, axis=-1, dtype=np.float32)  # [K]
    ct_aug = np.concatenate(
        [centers.T, (-0.5 * csq)[None, :]], axis=0
    ).astype(np.float32)  # [65, K]
    csqrep = np.ascontiguousarray(
        np.broadcast_to(csq, (TILE, K))
    ).astype(np.float32)
    revk = (256.0 - np.arange(K, dtype=np.float32)).astype(np.float32)
    revkrep = np.ascontiguousarray(np.broadcast_to(revk, (TILE, K)))

    zs = z.reshape(NCORES, npc, D)
    ones_row = np.ones((1, npc), dtype=np.float32)
    in_maps = []
    for c in range(NCORES):
        zt_aug = np.concatenate([zs[c].T, ones_row], axis=0)
        in_maps.append(
            {
                "zt": np.ascontiguousarray(zt_aug),
                "ct": ct_aug,
                "csqrep": csqrep,
                "revkrep": revkrep,
            }
        )
    return in_maps


def postprocess(results, weights, ntot=N):
    """Per-core device outputs -> full (cos_dist, new_weights, assignments)."""
    cos_parts, idx_parts = [], []
    counts = np.zeros(K, dtype=np.float32)
    for c in range(NCORES):
        r = results[c]
        # buffer[p, t] holds sample n = t*128 + p -> transpose to n-order
        cos_parts.append(np.ascontiguousarray(r["cosd"].T).reshape(-1))
        idx_parts.append(np.ascontiguousarray(r["idxout"].T).reshape(-1))
        counts += r["cnt"][0] / (256.0 - np.arange(K, dtype=np.float32))
    cos_dist = np.concatenate(cos_parts).astype(np.float32)
    assignments = np.concatenate(idx_parts).astype(np.int32)
    new_weights = (
        (1.0 - ALPHA) * np.asarray(weights, dtype=np.float32)
        + ALPHA * (counts / np.float32(ntot))
    ).astype(np.float32)
    return cos_dist, new_weights, assignments


def kernel(z, centers, weights):
    nc = _get_nc()
    in_maps = make_inputs(z, centers, weights)
    res = run_bass_kernel_spmd(nc, in_maps, list(range(NCORES)), trace=False)
    return postprocess(res.results, weights)
